# revision 8
# baseline (speedup 1.0000x reference)
"""Trainium2 Bass kernel for nn_ContrastiveLoss (segment_reduce).

Strategy (data-parallel over batch, 2 samples per core on 8 cores):
  - Host: normalize emb_q per pixel, transpose to pixel-major, cast fp8e4;
    pad each pixel to 20 cols with a trailing 1.0 (counts column).  Labels
    go to the device as bf16 (0..18, 255=ignore).  Shard by batch.
  - Device per core, per sample: stream tiles of 65536 pixels
    (zn [128, 512*20] fp8 on the SP HWDGE ring, labels [128, 512] bf16 on
    the ACT ring).  Build the one-hot mask on-device with one DVE is_equal
    per tile (labels broadcast against an iota row, fp8 output, pixel-major
    so fast-weight-load stays available).  Segment-reduce via PE matmuls
    with 6 pixel-chunks packed per instruction:
      lhsT = mask[:, 114j:114j+128]  (6x19 mask cols + 14 overlap cols ->
             exactly 128 weight cols => FWL engages)
      rhs  = zn[:, 120j:120j+120]    (6 chunks x 20 cols)
    accumulated in PSUM over the whole sample.  Diagonal 19x20 blocks of
    the [128,120] PSUM hold per-chunk-group class sums + counts; the rest
    is garbage that is never read.
  - Host: sum the 6 diagonal blocks, then per-sample means -> logits vs
    normalized emb_k -> log_softmax -> masked CE mean -> scalar loss.
"""

import os
import numpy as np
import ml_dtypes

import concourse.bass as bass
import concourse.mybir as mybir
import concourse.tile as tile
from concourse.bass_utils import run_bass_kernel_spmd

# ---------------------------------------------------------------- constants
N_CLASSES = 19
K = N_CLASSES + 1          # 20: 19 classes + counts column
TAU = 0.1
B, C, H, W = 16, 19, 512, 512
HW = H * W                 # 262144
NCORES = 8
SPC = B // NCORES          # samples per core = 2
P = 128                    # partitions / pixels per matmul chunk
G = 512                    # chunks per tile -> tile covers P*G = 65536 pixels
T = HW // (P * G)          # tiles per sample = 4
NPACK = G // 6             # 85 full 6-chunk groups per tile
NREM = G - NPACK * 6       # 2 leftover chunks per tile
F32 = mybir.dt.float32
BF16 = mybir.dt.bfloat16
FP8 = mybir.dt.float8e4
NP_FP8 = ml_dtypes.float8_e4m3

# ----------------------------------------------------- sync-wait splitting
# The walrus build in this container rejects instructions carrying more than
# ONE sync wait ("Too many sync wait commands").  Tile's scheduler freely
# attaches several waits to one instruction.  Post-process the BIR: move
# excess waits onto same-engine NOPs inserted immediately before.
def _split_sync_waits(nc, maxw=1):
    for f in nc.m.functions:
        for bb in f.blocks:
            newl = []
            changed = False
            for ins in bb.instructions:
                si = ins.sync_info
                w = list(si.on_wait) if si is not None else []
                if len(w) > maxw:
                    extra = w[:-maxw]
                    for j in range(0, len(extra), maxw):
                        grp = extra[j : j + maxw]
                        nop = mybir.InstNoOp(
                            name=f"{ins.name}_wsplit{j}", ins=[], outs=[]
                        )
                        nop.engine = ins.engine
                        nop.sync_info = mybir.SyncInfo(on_wait=grp, on_update=[])
                        newl.append(nop)
                    ins.sync_info = mybir.SyncInfo(
                        on_wait=w[-maxw:], on_update=list(si.on_update)
                    )
                    changed = True
                newl.append(ins)
            if changed:
                bb.instructions = newl


# ------------------------------------------------------------ device kernel
def _build_nc():
    nc = bass.Bass()
    zn = nc.dram_tensor("zn", [SPC * HW, K], FP8, kind="ExternalInput")
    lab = nc.dram_tensor("lab", [SPC * HW, 1], BF16, kind="ExternalInput")
    iota = nc.dram_tensor("iota", [P, N_CLASSES], BF16, kind="ExternalInput")
    out = nc.dram_tensor("out", [SPC, 114, 120], F32, kind="ExternalOutput")

    # pixel index = ((s*T + t)*P + p)*G + g
    zn_v = zn[:, :].rearrange("(s t p g) c -> s t p (g c)", s=SPC, t=T, p=P, g=G)
    lab_v = lab[:, :].rearrange("(s t p g) o -> s t p (g o)", s=SPC, t=T, p=P, g=G)

    with tile.TileContext(nc) as tc:
        with (
            tc.tile_pool(name="const", bufs=1) as cpool,
            tc.tile_pool(name="sbuf", bufs=3) as pool,
            tc.tile_pool(name="psum", bufs=2, space="PSUM") as ppool,
            tc.tile_pool(name="res", bufs=2) as rpool,
        ):
            iota_t = cpool.tile([P, N_CLASSES], BF16)
            nc.sync.dma_start(iota_t[:], iota[:, :])

            for s in range(SPC):
                acc = ppool.tile([P, 120], F32)
                for t_ in range(T):
                    zn_t = pool.tile([P, G * K], FP8, tag="zn")
                    lab_t = pool.tile([P, G], BF16, tag="lab")
                    # balance both HWDGE rings (sync->SP, scalar->ACT): zn is
                    # 10x the label bytes, so split it across the rings
                    half = G * K // 2
                    nc.sync.dma_start(zn_t[:, :half], zn_v[s, t_][:, :half])
                    nc.scalar.dma_start(zn_t[:, half:], zn_v[s, t_][:, half:])
                    nc.sync.dma_start(lab_t[:, : G // 2], lab_v[s, t_][:, : G // 2])
                    nc.scalar.dma_start(lab_t[:, G // 2 :], lab_v[s, t_][:, G // 2 :])

                    # one-hot mask, pixel-major fp8 (keeps FWL weight slices
                    # contiguous): mask[p, g, k] = (lab[p, g] == k).
                    # Two halves so the tile's matmuls can start before the
                    # whole mask is built (same total DVE work).
                    msk_t = pool.tile([P, G * N_CLASSES], FP8, tag="msk")
                    msk3 = msk_t[:].rearrange("p (g k) -> p g k", k=N_CLASSES)
                    GH = G // 2
                    for h in range(2):
                        nc.vector.tensor_tensor(
                            out=msk3[:, h * GH : (h + 1) * GH, :],
                            in0=lab_t[:, h * GH : (h + 1) * GH, None].to_broadcast(
                                [P, GH, N_CLASSES]
                            ),
                            in1=iota_t[:, None, :].to_broadcast([P, GH, N_CLASSES]),
                            op=mybir.AluOpType.is_equal,
                        )

                    for j in range(NPACK):
                        nc.tensor.matmul(
                            out=acc[:, :],
                            lhsT=msk_t[:, 114 * j : 114 * j + 128],
                            rhs=zn_t[:, 120 * j : 120 * j + 120],
                            start=(t_ == 0 and j == 0),
                            stop=False,
                            skip_group_check=True,
                        )
                    # leftover 2 chunks (510, 511) of this tile
                    nc.tensor.matmul(
                        out=acc[0 : NREM * N_CLASSES, 0 : NREM * K],
                        lhsT=msk_t[:, 114 * NPACK :],
                        rhs=zn_t[:, 120 * NPACK :],
                        start=False,
                        stop=(t_ == T - 1),
                        skip_group_check=True,
                    )

                res = rpool.tile([P, 120], F32)
                nc.vector.tensor_copy(res[0:114, :], acc[0:114, :])
                nc.sync.dma_start(out[s, :, :], res[0:114, :])

    _split_sync_waits(nc)
    return nc


_NC = None
LAST_RESULTS = None


def _get_nc():
    global _NC
    if _NC is None:
        _NC = _build_nc()
    return _NC


# --------------------------------------------------------------- host entry
def _make_in_maps(inputs):
    emb_q = np.asarray(inputs["emb_q"], dtype=np.float32)
    labels_np = np.asarray(inputs["labels"])

    # pixel-major normalized features, padded with a ones column, fp8
    feat = np.ascontiguousarray(
        emb_q.transpose(0, 2, 3, 1).reshape(B, HW, C)
    )
    nrm = np.sqrt(np.einsum("bpc,bpc->bp", feat, feat))
    np.maximum(nrm, 1e-12, out=nrm)
    zn_full = np.empty((B, HW, K), dtype=NP_FP8)
    zn_full[:, :, :C] = (feat / nrm[:, :, None]).astype(NP_FP8)
    zn_full[:, :, C] = NP_FP8(1.0)

    # labels as bf16 (0..18 and 255 are exact); 255 matches no iota column
    lab_full = labels_np.reshape(B, HW).astype(ml_dtypes.bfloat16)
    iota_np = np.ascontiguousarray(
        np.broadcast_to(
            np.arange(N_CLASSES, dtype=ml_dtypes.bfloat16), (P, N_CLASSES)
        )
    )

    in_maps = []
    for i in range(NCORES):
        in_maps.append(
            {
                "zn": zn_full[i * SPC : (i + 1) * SPC].reshape(SPC * HW, K),
                "lab": lab_full[i * SPC : (i + 1) * SPC].reshape(SPC * HW, 1),
                "iota": iota_np,
            }
        )
    return in_maps


def kernel(emb_k, emb_q, labels, epoch):
    emb_k = np.asarray(emb_k, dtype=np.float32)
    epoch_val = int(np.asarray(epoch))
    in_maps = _make_in_maps({"emb_q": emb_q, "labels": labels})

    nc = _get_nc()
    res = run_bass_kernel_spmd(
        nc,
        in_maps,
        core_ids=list(range(NCORES)),
        trace=bool(int(os.environ.get("KERNEL_TRACE", "0"))),
    )
    global LAST_RESULTS
    LAST_RESULTS = res

    # [16, 114, 120]: six diagonal 19x20 blocks hold (sums | count) partials
    outs = np.concatenate([r["out"] for r in res.results], axis=0)
    tot = np.zeros((B, N_CLASSES, K), dtype=np.float32)
    for j in range(6):
        tot += outs[:, 19 * j : 19 * j + 19, 20 * j : 20 * j + 20]
    sums = tot[:, :, :N_CLASSES]
    counts = tot[:, :, N_CLASSES]

    # tiny CE epilogue in f32, mirroring the reference
    ekn = emb_k / np.maximum(
        np.linalg.norm(emb_k, axis=-1, keepdims=True), 1e-12
    ).astype(np.float32)
    means = sums / np.maximum(counts, 1.0)[:, :, None]          # [B, 19, 19]
    logits = np.einsum("bkc,nc->bkn", means, ekn).astype(np.float32) / np.float32(TAU)
    m = logits.max(axis=-1, keepdims=True)
    shifted = logits - m
    logp = shifted - np.log(np.exp(shifted).sum(axis=-1, keepdims=True))
    ce = -np.einsum("bkk->bk", logp)                            # diag, [B, 19]
    valid = counts > 0.0
    nvalid = valid.sum(axis=-1).astype(np.float32)
    per_sample = (ce * valid).sum(axis=-1) / np.maximum(nvalid, 1.0)
    total = np.where(nvalid > 0, per_sample, 0.0).sum() / np.float32(B)
    result = np.float32(total) if epoch_val != 0 else np.float32(0.0)
    return np.asarray(result, dtype=np.float32)
